# revision 7
# baseline (speedup 1.0000x reference)
# Trainium2 Bass kernel for nn_CN_MLP_71631464563230 (moe_routing).
#
# Math: the reference is
#   mo = x @ W.T + b;  w = softmax(mo @ attn);  out = sigmoid(w . (mo @ V.T) + cla_b)
# with V[t,h] = sum_k CM[t,h,k] cla_w[k]. Both pre-softmax quantities are
# LINEAR in mo, and mo is affine in x, so by associativity
#   a  = mo @ attn = x @ (W.T @ attn) + (b @ attn)
#   s  = mo @ V.T  = x @ (W.T @ V.T)  + (V @ b)
# The parameter-only folds G = [W.T attn | W.T V.T] (D x 2T), ca, cs are
# precomputed on the host at pack time (constant folding through linear
# layers, like BN-into-conv). The device computes asT = G.T x.T per core
# (fp8 DoubleRow, 16x fewer MACs than materializing mo) and the nonlinear
# epilogue:  out = sigmoid( (sum_t e^{a} s)/(sum_t e^{a}) + cla_b ).
# CM / mlp_w / attn never need to be DMA'd: per-core traffic drops from
# 16.4MB to 5.4MB, and the old ~40us DVE V-fold disappears entirely.
#
# Sharding: batch 8x data-parallel (1024 rows/core); G + consts replicated
# (G is 320KB fp8). G is scaled by 64 host-side to clear the e4m3 denormal
# floor (G elems ~ N(0, 1/5000)); the scale is inverted in the ACT scale
# operand of the two Exp ops, and cs is pre-multiplied by it. a lands in
# PSUM partitions 0:16 and s in 32:48 (DVE reads of PSUM must be
# 32-partition-quadrant aligned). Only the Exp ACT table is ever loaded.
#
# DMA reality: each dma_start costs ~700ns of trigger time on its issuing
# engine and partition lines under ~4KB drop throughput, so x streams in 9
# big transfers (5KB lines) on the sync queue while G + consts go on the
# scalar queue in parallel. The batch is split into 4 groups of 256 so
# each group's epilogue overlaps the next group's stream; the last group
# tapers (20/14/6 k-tiles) so little matmul work trails the final byte.
# E and P share one tile so den|num is a single 16-row-sum matmul; each
# group DMAs its own output slice out as soon as it is ready.

import os

import ml_dtypes
import numpy as np

import concourse.bass as bass
import concourse.mybir as mybir
import concourse.tile as tile
from concourse import bacc
from concourse.bass_utils import run_bass_kernel_spmd

B, D, H, T = 8192, 5000, 512, 16
NCORES = 8
BLOC = B // NCORES            # 1024 batch rows per core
KT = (D + 127) // 128         # 40 k-tiles over D (last padded)
KP = KT // 2                  # 20 DoubleRow k-pairs
NG = 4                        # batch column groups per core
GB = BLOC // NG               # 256 batch rows per group
CHUNKS = [[(0, 20), (20, 40)]] * 3 + [[(0, 20), (20, 34), (34, 40)]]
G_SCALE = 64.0
M2 = 64                       # PE out partitions: a at 0:16, s at 32:48

F32 = mybir.dt.float32
BF = mybir.dt.bfloat16
F8 = mybir.dt.float8e4
NP_F8 = mybir.dt.np(F8)       # ml_dtypes.float8_e4m3 (TRN semantics, max 240)
DR = mybir.MatmulPerfMode.DoubleRow
AF = mybir.ActivationFunctionType

LAST_RESULTS = None


def _build_nc():
    nc = bacc.Bacc("TRN2", target_bir_lowering=False)

    xT = nc.dram_tensor("xT", [128, NG * KT * GB], F8, kind="ExternalInput").ap()
    gT = nc.dram_tensor("gT", [128, KT * M2], F8, kind="ExternalInput").ap()
    ca_d = nc.dram_tensor("ca", [T, 1], F32, kind="ExternalInput").ap()
    cs_d = nc.dram_tensor("csp", [T, 1], F32, kind="ExternalInput").ap()
    clabn = nc.dram_tensor("clabn", [1, 1], F32, kind="ExternalInput").ap()
    out_d = nc.dram_tensor("out", [1, BLOC], F32, kind="ExternalOutput").ap()

    with tile.TileContext(nc) as tc:
        import contextlib

        ctx = contextlib.ExitStack()
        with ctx:
            sg = ctx.enter_context(tc.tile_pool(name="sg", bufs=1))
            pp = ctx.enter_context(tc.tile_pool(name="pp", bufs=1, space="PSUM"))

            # ---- tiles ----------------------------------------------------
            G_sb = sg.tile([128, KT, M2], F8, tag="G_sb")
            xch = {}
            for g in range(NG):
                for (k0, k1) in CHUNKS[g]:
                    xch[g, k0] = sg.tile([128, k1 - k0, GB], F8,
                                         tag=f"x{g}_{k0}", name=f"x{g}_{k0}")
            ca_sb = sg.tile([T, 1], F32, tag="ca_sb")
            cs_sb = sg.tile([T, 1], F32, tag="cs_sb")
            clabn_sb = sg.tile([1, 1], F32, tag="clabn_sb")
            ones16 = sg.tile([T, 1], BF, tag="ones16")
            warm1 = sg.tile([1, 1], F32, tag="warm1")
            # E and P adjacent so den|num is one 16-row-sum matmul; every
            # group gets its OWN tiles — slices of one shared tile create
            # false WAR deps that lockstep-serialize the group chains
            EP_sb = [sg.tile([T, 2, GB], BF, tag=f"EP{g}", name=f"EP{g}")
                     for g in range(NG)]
            rden = [sg.tile([1, GB], F32, tag=f"rden{g}", name=f"rden{g}")
                    for g in range(NG)]
            lg = [sg.tile([1, GB], F32, tag=f"lg{g}", name=f"lg{g}")
                  for g in range(NG)]
            eneg = [sg.tile([1, GB], F32, tag=f"eneg{g}", name=f"eneg{g}")
                    for g in range(NG)]
            ep1 = [sg.tile([1, GB], F32, tag=f"ep1_{g}", name=f"ep1_{g}")
                   for g in range(NG)]
            orow = [sg.tile([1, GB], F32, tag=f"orow{g}", name=f"orow{g}")
                    for g in range(NG)]

            mm_ps = [pp.tile([M2, GB], F32, tag=f"p{g}", name=f"mm{g}")
                     for g in range(NG)]

            # ---- x stream on the sync queue, big transfers ----------------
            for g in range(NG):
                for (k0, k1) in CHUNKS[g]:
                    lo = (g * KT + k0) * GB
                    nc.sync.dma_start(
                        out=xch[g, k0].rearrange("p k b -> p (k b)"),
                        in_=xT[:, lo:lo + (k1 - k0) * GB])

            # ---- G + consts on the scalar queue; Exp table preload --------
            nc.scalar.dma_start(
                out=G_sb.rearrange("p k m -> p (k m)"), in_=gT)
            nc.scalar.dma_start(out=ca_sb, in_=ca_d)
            nc.scalar.dma_start(out=cs_sb, in_=cs_d)
            nc.scalar.dma_start(out=clabn_sb, in_=clabn)
            nc.gpsimd.memset(ones16, 1.0)
            nc.scalar.activation(warm1, clabn_sb, AF.Exp)

            # ---- per-group matmuls and 3-stage epilogue, software-pipelined
            # across groups: each engine's issue order matches readiness
            # order, so a later group's ready op is never stuck behind an
            # earlier group's not-yet-ready one
            def mms(g):
                for (k0, k1) in CHUNKS[g]:
                    xt = xch[g, k0]
                    for kp in range(k0 // 2, k1 // 2):
                        lk = 2 * kp - k0
                        nc.tensor.matmul(
                            mm_ps[g],
                            lhsT=G_sb[:, 2 * kp:2 * kp + 2, :],
                            rhs=xt[:, lk:lk + 2, :],
                            start=(kp == 0), stop=(kp == KP - 1),
                            perf_mode=DR)

            def stage_a(g):
                # E = exp(a/G_SCALE + ca);  P = (s + G_SCALE*cs) * E;
                # den|num = ones16.T @ [E | P]  in one matmul
                nc.scalar.activation(EP_sb[g][:, 0, :], mm_ps[g][0:T, :],
                                     AF.Exp, bias=ca_sb, scale=1.0 / G_SCALE)
                nc.vector.scalar_tensor_tensor(
                    out=EP_sb[g][:, 1, :], in0=mm_ps[g][32:48, :],
                    scalar=cs_sb, in1=EP_sb[g][:, 0, :],
                    op0=mybir.AluOpType.add, op1=mybir.AluOpType.mult)
                dn_ps = pp.tile([1, 2 * GB], F32, tag=f"p{4 + g}",
                                name=f"dn{g}")
                nc.tensor.matmul(
                    dn_ps, lhsT=ones16,
                    rhs=EP_sb[g].rearrange("t a b -> t (a b)"),
                    start=True, stop=True)
                return dn_ps

            def stage_b(g, dn_ps):
                nc.vector.reciprocal_approx_fast(
                    out=rden[g], in_=dn_ps[:, 0:GB])
                nc.vector.tensor_mul(lg[g], dn_ps[:, GB:2 * GB], rden[g])

            def stage_c(g):
                # sigmoid(num/den/G_SCALE + cla_b) via Exp + fast reciprocal
                nc.scalar.activation(eneg[g], lg[g], AF.Exp,
                                     bias=clabn_sb, scale=-1.0 / G_SCALE)
                nc.vector.tensor_scalar_add(ep1[g], eneg[g], 1.0)
                nc.vector.reciprocal_approx_fast(out=orow[g], in_=ep1[g])
                nc.sync.dma_start(out=out_d[:, g * GB:(g + 1) * GB],
                                  in_=orow[g])

            for g in range(NG):
                mms(g)
                stage_b(g, stage_a(g))
                stage_c(g)

    nc.finalize()
    return nc


_NC_CACHE = None


def _pack_inputs(data_input, mlp_w, mlp_b, CM, attn, cla_w, cla_b):
    x = np.asarray(data_input, dtype=np.float32)
    W = np.asarray(mlp_w, dtype=np.float32)
    b = np.asarray(mlp_b, dtype=np.float32)
    CM = np.asarray(CM, dtype=np.float32)
    attn = np.asarray(attn, dtype=np.float32)
    cla_w = np.asarray(cla_w, dtype=np.float32).reshape(H)
    cla_b = np.asarray(cla_b, dtype=np.float32).reshape(1, 1)

    # Parameter folds (host, O(D*H) — data-independent)
    V = CM @ cla_w                       # [T, H]
    Ga = W.T @ attn                      # [D, T]
    Gs = W.T @ V.T                       # [D, T]
    ca = (b @ attn).reshape(T, 1)
    csp = (G_SCALE * (V @ b)).reshape(T, 1)

    DP = KT * 128
    # x: [B, D] -> per core [128, (g kt j)] fp8, group-major
    xp = np.zeros((B, DP), dtype=np.float32)
    xp[:, :D] = np.clip(x, -240, 240)
    xp = (xp.reshape(NCORES, NG, GB, KT, 128)
            .transpose(0, 4, 1, 3, 2)        # [core, 128, g, kt, j]
            .reshape(NCORES, 128, NG * KT * GB)
            .astype(NP_F8))
    # G: [D, 2T] -> [128, (kt m)] fp8, scaled, quadrant-padded
    gp = np.zeros((DP, M2), dtype=np.float32)
    gp[:D, 0:T] = np.clip(Ga * G_SCALE, -240, 240)
    gp[:D, 32:32 + T] = np.clip(Gs * G_SCALE, -240, 240)
    gp = (gp.reshape(KT, 128, M2).transpose(1, 0, 2)
            .reshape(128, KT * M2).astype(NP_F8))

    shared = {"gT": gp, "ca": np.ascontiguousarray(ca),
              "csp": np.ascontiguousarray(csp),
              "clabn": np.ascontiguousarray(-cla_b)}
    return [
        {"xT": np.ascontiguousarray(xp[i]), **shared}
        for i in range(NCORES)
    ]


def kernel(data_input, mlp_w, mlp_b, CM, attn, cla_w, cla_b):
    global LAST_RESULTS, _NC_CACHE

    in_maps = _pack_inputs(data_input, mlp_w, mlp_b, CM, attn, cla_w, cla_b)

    if _NC_CACHE is None:
        _NC_CACHE = _build_nc()

    trace = bool(int(os.environ.get("KERNEL_TRACE", "0")))
    res = run_bass_kernel_spmd(
        _NC_CACHE, in_maps, core_ids=list(range(NCORES)), trace=trace,
        trace_cores=[0] if trace else None,
    )
    LAST_RESULTS = res

    full = np.empty(B, dtype=np.float32)
    for i in range(NCORES):
        full[i * BLOC:(i + 1) * BLOC] = res.results[i]["out"].reshape(BLOC)
    return full
